# revision 29
# baseline (speedup 1.0000x reference)
"""Trainium2 Bass kernel: Brevitas-style int4 fake-quant Conv2d (3x3, pad 1).

reference:
    wq = fake_quant_per_channel(w)          # per-O-channel int4 scale
    out = conv2d(x, wq, NCHW/OIHW, pad 1)

Strategy (v3, 1D Winograd F(2,3) along H):
  * Host: fake-quant w -> wq (f32), fold the Winograd weight transform G
    (and the per-channel scale, already inside wq) into
    U[u, o, c, kw] = sum_r G[u, r] wq[o, c, r, kw], cast fp16.
  * Device (data-parallel, 4 images/core x 8 cores): per image build a
    padded fp16 xp [128, 2kt, 58, 58], then 4 Winograd input planes
    V_u[c, t, col] = B^T[u, :] . xp[c, 2t:2t+4, col] (t = 28 row tiles):
      V0 = r0 - r2   V1 = r1 + r2   V2 = r2 - r1   V3 = r1 - r3
    -- 4 pure tensor_tensor ops (2x_1p DVE mode; scalar_tensor_tensor
    would be 1x-only).  The conv becomes, per (ot, chunk of 7 tiles):
    4 groups M_u [128o, 7, 56] = sum_{kt,kw} U[u,kw,kt,ot].T @ V_u[kt]
    -- 24 matmuls of 392 cols vs 36 direct-conv equivalents: 1.5x fewer
    PE cycles than the direct implicit GEMM.
  * Output transform (A^T = [[1,1,1,0],[0,1,-1,-1]]) during the drain:
      c2=copy(m2) [ACT]  t1=m1+c2, t2=m1-c2 [DVE, 1 PSUM operand each]
      c0=copy(m0), c3=copy(m3) [ACT]  o0=t1+c0, o1=t2-c3 [gpsimd]
    (TT is DVE/Pool-only on trn2; Pool has no PSUM port; TT reads at
    most one PSUM operand -> ACT activation-copies bridge the gap.)
    Output rows interleave 2t+j into the f32 out tile, DMA per chunk.
  * Accuracy: fp16 x/V/U with f32 PSUM — host-sim of this pipeline
    measures absmax rel err ~4e-4 vs the f32 reference (gate 2e-2).
"""

import os
import sys
from contextlib import ExitStack

for _p in ("/opt/trn_rl_repo", "/root/.axon_site/_ro/trn_rl_repo"):
    if os.path.isdir(_p) and _p not in sys.path:
        sys.path.insert(0, _p)

import numpy as np

import concourse.bass as bass  # noqa: F401
import concourse.mybir as mybir
import concourse.tile as tile
from concourse import bacc
from concourse.bass_utils import run_bass_kernel_spmd

F32 = mybir.dt.float32
FP16 = mybir.dt.float16

# Problem shapes (hardcoded per contract).
N, C, H, W = 32, 256, 56, 56
O, KH, KW = 256, 3, 3
CORES = 8
NPC = N // CORES  # images per core

QMAX = 7.0
SCALING_MIN_VAL = 2e-16

KT = C // 128
OT = O // 128
NU = 4                 # winograd taps
TW = H // 2            # 28 winograd row tiles
TR = 7                 # tiles per chunk
NCI = TW // TR         # 4 chunks per (img, ot)
HP, WP = 58, 58        # padded rows/cols
NSTRIP = 7             # x DMA strips of 8 rows
U_ORDER = (1, 2, 0, 3)
PLANE = HP * WP        # fp16 elems per kt plane

# F(2,3): G (weight transform).  B^T/A^T are hardcoded in the op lists.
G_MAT = np.array([
    [1, 0, 0],
    [1 / 2, 1 / 2, 1 / 2],
    [1 / 2, -1 / 2, 1 / 2],
    [0, 0, 1],
], dtype=np.float64)


def build_nc(npc=NPC, warmup_mms=40):
    nc = bacc.Bacc("TRN2", target_bir_lowering=False, debug=False)
    x_d = nc.dram_tensor("x", [npc, C, H, W], F32, kind="ExternalInput").ap()
    w_d = nc.dram_tensor("wu", [128, NU * 3 * KT * OT * 128], FP16,
                         kind="ExternalInput").ap()
    out_d = nc.dram_tensor("out", [npc, O, H, W], F32,
                           kind="ExternalOutput").ap()

    with tile.TileContext(nc) as tc, ExitStack() as ctx:
        wpool = ctx.enter_context(tc.tile_pool(name="wpool", bufs=1))
        xspool = ctx.enter_context(tc.tile_pool(name="xspool", bufs=30))
        hpool = ctx.enter_context(tc.tile_pool(name="hpool", bufs=2))
        vpool = ctx.enter_context(tc.tile_pool(name="vpool", bufs=2))
        s32pool = ctx.enter_context(tc.tile_pool(name="s32pool", bufs=10))
        opool = ctx.enter_context(tc.tile_pool(name="opool", bufs=6))
        ppool = ctx.enter_context(tc.tile_pool(name="ppool", bufs=8,
                                               space="PSUM"))

        w_sb = wpool.tile([128, NU * 3 * KT * OT * 128], FP16)
        # two sequential transfers on the same queue: the first covers taps
        # u=0,1,2 so the first accumulation groups unblock ~4us earlier
        WA = 3 * 3 * KT * OT * 128
        nc.scalar.dma_start(w_sb[:, :WA], w_d[:, :WA])
        nc.scalar.dma_start(w_sb[:, WA:], w_d[:, WA:])

        def wslice(u, kw, kt, ot):
            j = (((u * 3 + kw) * KT + kt) * OT + ot) * 128
            return w_sb[:, j:j + 128]

        if warmup_mms:
            # steady-state-shaped warmup: FD=392 MMs (LDW per MM, like the
            # real stream); memset on gpsimd, whose program loads earliest
            wu = wpool.tile([128, 392], FP16)
            nc.gpsimd.memset(wu[:, :], 0.0)
            wu_ps = ppool.tile([128, 512], F32, tag="ps", name="wu_ps")
            for i in range(warmup_mms):
                nc.tensor.matmul(wu_ps[:, :392], wu[:, :128],
                                 wu[:, :].rearrange(
                                     "p (t c) -> p t c", c=56),
                                 start=True, stop=True)

        xp_t = {}  # img -> xp tile ([128, 2*PLANE] fp16)
        vv = {}    # img -> [128, 2, NU, TW, WP] view

        def emit_pad(img):
            xp = hpool.tile([128, KT * PLANE], FP16, tag="xp")
            xp_t[img] = xp
            for kt in range(KT):
                b = kt * PLANE
                nc.vector.memset(xp[:, b: b + WP + 1], 0.0)
                nc.vector.memset(xp[:, b + (HP - 1) * WP - 1: b + HP * WP],
                                 0.0)
                nc.vector.memset(
                    xp[:, b + 2 * WP - 1: b + 2 * WP - 1 + (HP - 3) * WP]
                    .rearrange("p (a b) -> p a b", b=WP)[:, :, 0:2], 0.0)

        def emit_strip_triggers(img, s0, s1, all_sync=False):
            tiles = []
            for kt in range(KT):
                for s in range(s0, s1):
                    xs = xspool.tile([128, 8, W], F32, tag="xs")
                    q = nc.sync if (kt == 0 or all_sync) else nc.scalar
                    q.dma_start(xs[:, :, :],
                                x_d[img, kt * 128:(kt + 1) * 128,
                                    8 * s: 8 * s + 8, :])
                    tiles.append((kt, s, xs))
            return tiles

        def emit_converts(img, tiles, eng=None):
            v = xp_t[img][:, :].rearrange("p (k r c) -> p k r c", k=KT, c=WP)
            for kt, s, xs in tiles:
                dst = v[:, kt, 1 + 8 * s: 9 + 8 * s, 1:1 + W]
                if eng == "dve":
                    nc.vector.tensor_copy(dst, xs[:, :, :])
                else:
                    nc.scalar.copy(dst, xs[:, :, :])

        V_OPS = {1: ("add", 1, 2), 2: ("sub", 2, 1),
                 0: ("sub", 0, 2), 3: ("sub", 1, 3)}

        def emit_V(img, half, per_kt=False, us=(1, 2, 0, 3)):
            """V planes (taps in `us`) for tiles [half*14, half*14+14)."""
            if img not in vv:
                vt = vpool.tile([128, KT * NU * TW * WP], FP16, tag="v")
                vv[img] = vt[:, :].rearrange(
                    "p (k u t c) -> p k u t c", k=KT, u=NU, c=WP)
            v5 = vv[img]
            ta, tb = half * 14, half * 14 + 14
            xp2 = xp_t[img][:, :].rearrange(
                "p (k t f c) -> p k t f c", k=KT, f=2, c=WP)

            kts = [(kt, kt + 1) for kt in range(KT)] if per_kt \
                else [(0, KT)]

            def R(ka, kb, i):
                q, rr = divmod(i, 2)
                return xp2[:, ka:kb, ta + q: tb + q, rr, :]

            # op-major across kt, in U_ORDER, so img-0 MM groups (which
            # consume both kt planes of one u) start as early as possible
            for u in us:
                kind, i0, i1 = V_OPS[u]
                op = nc.vector.tensor_add if kind == "add" \
                    else nc.vector.tensor_sub
                for ka, kb in kts:
                    op(v5[:, ka:kb, u, ta:tb, :],
                       R(ka, kb, i0), R(ka, kb, i1))

        def s32():
            t = s32pool.tile([128, TR * W], F32, tag="s32")
            return t[:, :].rearrange("p (t c) -> p t c", c=W)

        def emit_chunk(img, ci, ot, last=False, v_hook=None):
            m = {}
            ob = opool.tile([128, 2 * TR * W], F32, tag="ob")
            ob2 = ob[:, :].rearrange("p (t f c) -> p t f c", f=2, c=W)
            t1 = t2 = None
            dst = out_d[img, ot * 128:(ot + 1) * 128,
                        2 * TR * ci: 2 * TR * (ci + 1), :]
            dst2 = dst.rearrange("p (t f) c -> p t f c", f=2)
            for u in ((1, 2, 3, 0) if last else U_ORDER):
                if v_hook is not None:
                    v_hook(u)
                ps = ppool.tile([128, 512], F32, tag="ps", name=f"ps{u}")
                mv = ps[:, : TR * W].rearrange("p (t c) -> p t c", c=W)
                m[u] = mv
                idx = 0
                for kt in range(KT):
                    vsl = vv[img][:, kt, u, ci * TR:(ci + 1) * TR, :]
                    for kw in range(3):
                        nc.tensor.matmul(
                            mv[:, :, :], wslice(u, kw, kt, ot),
                            vsl[:, :, kw: kw + W],
                            start=(idx == 0), stop=(idx == 3 * KT - 1),
                        )
                        idx += 1
                # drains trail each accumulation group. TT is DVE/Pool-only,
                # Pool can't read PSUM, TT reads <=1 PSUM operand, so ACT
                # activation-copies bridge PSUM->SBUF for the second inputs.
                if u == 2:
                    c2 = s32()
                    nc.scalar.copy(c2, m[2])
                    t1, t2 = s32(), s32()
                    nc.vector.tensor_add(t1, m[1], c2)   # m1 + m2
                    nc.vector.tensor_sub(t2, m[1], c2)   # m1 - m2
                if u == 3 and last:
                    c3 = s32()
                    nc.scalar.copy(c3, m[3])
                    nc.vector.tensor_sub(ob2[:, :, 1, :], t2, c3)
                    nc.sync.dma_start(dst2[:, :, 1, :], ob2[:, :, 1, :])
                if u == 0:
                    c0 = s32()
                    nc.scalar.copy(c0, m[0])
                    oeng = nc.vector if img == npc - 1 else nc.gpsimd
                    oeng.tensor_add(ob2[:, :, 0, :], t1, c0)
                    if last:
                        nc.sync.dma_start(dst2[:, :, 0, :], ob2[:, :, 0, :])
            if not last:
                c3 = s32()
                nc.scalar.copy(c3, m[3])
                oeng = nc.vector if img == npc - 1 else nc.gpsimd
                oeng.tensor_sub(ob2[:, :, 1, :], t2, c3)
                obv = ob[:, :].rearrange("p (r c) -> p r c", c=W)
                nc.sync.dma_start(dst[:, :, :], obv[:, :, :])

        # ---------------- schedule ----------------
        # In-order engines: DMA-paced prefetch ops must never be queued
        # ahead of ring-critical drain ops (ACT copies / DVE t1,t2), or the
        # PSUM ring stalls the PE.  So prefetch is sliced into small batches
        # emitted AFTER each chunk, each batch data-ready by its slot time.
        # image 0: kt1 converts ride gpsimd so both kt halves convert in
        # parallel; V per-kt so the first MM groups start early.
        emit_pad(0)
        st0 = emit_strip_triggers(0, 0, 4, all_sync=True)
        emit_converts(0, [t for t in st0 if t[0] == 0])
        emit_converts(0, [t for t in st0 if t[0] == 1], eng="dve")
        st1 = emit_strip_triggers(0, 4, NSTRIP, all_sync=True)

        CHUNKS = [(ci, ot) for ci in range(NCI) for ot in range(OT)]

        def cpick(strips, kt, ss):
            return [t for t in strips if t[0] == kt and t[1] in ss]

        for img in range(npc):
            nxt = img + 1
            if nxt < npc:
                emit_pad(nxt)
                strips = emit_strip_triggers(nxt, 0, NSTRIP)
            for qi, (ci, ot) in enumerate(CHUNKS):
                hook = None
                if img == 0 and qi == 0:
                    hook = lambda u: emit_V(0, 0, per_kt=True, us=(u,))
                emit_chunk(img, ci, ot,
                           last=(img == npc - 1 and qi == len(CHUNKS) - 1),
                           v_hook=hook)
                if img == 0:
                    # image 0's own half-1 input path
                    if qi == 0:
                        emit_converts(0, st1)
                    elif qi == 1:
                        emit_V(0, 1, us=(1, 2))
                    elif qi == 2:
                        emit_V(0, 1, us=(0, 3))
                    if nxt < npc:
                        if qi == 3:
                            emit_converts(nxt, cpick(strips, 0, (0, 1, 2)))
                        elif qi == 4:
                            emit_converts(nxt, cpick(strips, 0, (3, 4, 5)))
                        elif qi == 5:
                            emit_converts(nxt, cpick(strips, 0, (6,))
                                          + cpick(strips, 1, (0, 1, 2)))
                        elif qi == 6:
                            emit_converts(nxt, cpick(strips, 1, (3, 4, 5, 6)))
                            emit_V(nxt, 0, us=(1, 2))
                        elif qi == 7:
                            emit_V(nxt, 0, us=(0, 3))
                else:
                    # own half-1 V (its converts all done last window)
                    if qi == 0:
                        emit_V(img, 1, us=(1, 2))
                    elif qi == 1:
                        emit_V(img, 1, us=(0, 3))
                    if nxt < npc:
                        if qi == 2:
                            emit_converts(nxt, cpick(strips, 0, (0, 1, 2)))
                        elif qi == 3:
                            emit_converts(nxt, cpick(strips, 0, (3, 4, 5)))
                        elif qi == 4:
                            emit_converts(nxt, cpick(strips, 0, (6,))
                                          + cpick(strips, 1, (0, 1)))
                        elif qi == 5:
                            emit_converts(nxt, cpick(strips, 1, (2, 3, 4)))
                        elif qi == 6:
                            emit_converts(nxt, cpick(strips, 1, (5, 6)))
                            emit_V(nxt, 0, us=(1, 2))
                        elif qi == 7:
                            emit_V(nxt, 0, us=(0, 3))

    nc.compile()
    return nc


def quantize_weights(w):
    """Match reference fake-quant in f32: returns wq = dequantized weights."""
    w = np.asarray(w, np.float32)
    amax = np.max(np.abs(w), axis=(1, 2, 3), keepdims=True).astype(np.float32)
    scale = np.maximum((amax / np.float32(QMAX)).astype(np.float32),
                       np.float32(SCALING_MIN_VAL)).astype(np.float32)
    q = np.clip(np.rint((w / scale).astype(np.float32)),
                -QMAX, QMAX).astype(np.float32)
    return (q * scale).astype(np.float32)


def pack_weights(wq):
    """wq [O,C,3,3] -> winograd U packed [128, (u,kw,kt,ot,o_loc)] fp16."""
    u4 = np.einsum("ur,ocrk->uock", G_MAT,
                   wq.astype(np.float64)).astype(np.float32)
    a = u4.reshape(NU, OT, 128, KT, 128, 3)      # [u, ot, o, kt, c, kw]
    p = a.transpose(4, 0, 5, 3, 1, 2)            # [c, u, kw, kt, ot, o]
    return np.ascontiguousarray(p).reshape(
        128, NU * 3 * KT * OT * 128).astype(np.float16)


_nc_cache = {}
LAST_RESULT = None  # BassKernelResults of the most recent kernel() call


def kernel(x, w):
    global LAST_RESULT
    x = np.ascontiguousarray(np.asarray(x, np.float32))
    w = np.asarray(w, np.float32)
    assert x.shape == (N, C, H, W) and w.shape == (O, C, KH, KW)

    w_host = pack_weights(quantize_weights(w))

    if "nc" not in _nc_cache:
        _nc_cache["nc"] = build_nc()
    nc = _nc_cache["nc"]

    in_maps = [
        {"x": np.ascontiguousarray(x[cid * NPC:(cid + 1) * NPC]),
         "wu": w_host}
        for cid in range(CORES)
    ]
    kwargs = {}
    trace_dir = os.environ.get("KERNEL_TRACE_DIR")
    if trace_dir:  # dev-harness profiling only; unset in normal use
        kwargs = {"trace": True, "tmpdir": trace_dir}
    res = run_bass_kernel_spmd(nc, in_maps, list(range(CORES)), **kwargs)
    LAST_RESULT = res
    return np.concatenate([res.results[cid]["out"] for cid in range(CORES)],
                          axis=0)


if __name__ == "__main__":
    rng = np.random.default_rng(0)
    x = rng.standard_normal((N, C, H, W), dtype=np.float32)
    w = rng.standard_normal((O, C, KH, KW), dtype=np.float32) * 0.05
    out = kernel(x, w)
    print("out", out.shape, out.dtype, float(np.abs(out).max()))


# revision 30
# speedup vs baseline: 1.0175x; 1.0175x over previous
"""Trainium2 Bass kernel: Brevitas-style int4 fake-quant Conv2d (3x3, pad 1).

reference:
    wq = fake_quant_per_channel(w)          # per-O-channel int4 scale
    out = conv2d(x, wq, NCHW/OIHW, pad 1)

Strategy (v3, 1D Winograd F(2,3) along H):
  * Host: fake-quant w -> wq (f32), fold the Winograd weight transform G
    (and the per-channel scale, already inside wq) into
    U[u, o, c, kw] = sum_r G[u, r] wq[o, c, r, kw], cast fp16.
  * Device (data-parallel, 4 images/core x 8 cores): per image build a
    padded fp16 xp [128, 2kt, 58, 58], then 4 Winograd input planes
    V_u[c, t, col] = B^T[u, :] . xp[c, 2t:2t+4, col] (t = 28 row tiles):
      V0 = r0 - r2   V1 = r1 + r2   V2 = r2 - r1   V3 = r1 - r3
    -- 4 pure tensor_tensor ops (2x_1p DVE mode; scalar_tensor_tensor
    would be 1x-only).  The conv becomes, per (ot, chunk of 7 tiles):
    4 groups M_u [128o, 7, 56] = sum_{kt,kw} U[u,kw,kt,ot].T @ V_u[kt]
    -- 24 matmuls of 392 cols vs 36 direct-conv equivalents: 1.5x fewer
    PE cycles than the direct implicit GEMM.
  * Output transform (A^T = [[1,1,1,0],[0,1,-1,-1]]) during the drain:
      c2=copy(m2) [ACT]  t1=m1+c2, t2=m1-c2 [DVE, 1 PSUM operand each]
      c0=copy(m0), c3=copy(m3) [ACT]  o0=t1+c0, o1=t2-c3 [gpsimd]
    (TT is DVE/Pool-only on trn2; Pool has no PSUM port; TT reads at
    most one PSUM operand -> ACT activation-copies bridge the gap.)
    Output rows interleave 2t+j into the f32 out tile, DMA per chunk.
  * Accuracy: fp16 x/V/U with f32 PSUM — host-sim of this pipeline
    measures absmax rel err ~4e-4 vs the f32 reference (gate 2e-2).
"""

import os
import sys
from contextlib import ExitStack

for _p in ("/opt/trn_rl_repo", "/root/.axon_site/_ro/trn_rl_repo"):
    if os.path.isdir(_p) and _p not in sys.path:
        sys.path.insert(0, _p)

import numpy as np

import concourse.bass as bass  # noqa: F401
import concourse.mybir as mybir
import concourse.tile as tile
from concourse import bacc
from concourse.bass_utils import run_bass_kernel_spmd

F32 = mybir.dt.float32
FP16 = mybir.dt.float16

# Problem shapes (hardcoded per contract).
N, C, H, W = 32, 256, 56, 56
O, KH, KW = 256, 3, 3
CORES = 8
NPC = N // CORES  # images per core

QMAX = 7.0
SCALING_MIN_VAL = 2e-16

KT = C // 128
OT = O // 128
NU = 4                 # winograd taps
TW = H // 2            # 28 winograd row tiles
TR = 7                 # tiles per chunk
NCI = TW // TR         # 4 chunks per (img, ot)
HP, WP = 58, 58        # padded rows/cols
NSTRIP = 7             # x DMA strips of 8 rows
U_ORDER = (1, 2, 0, 3)
PLANE = HP * WP        # fp16 elems per kt plane

# F(2,3): G (weight transform).  B^T/A^T are hardcoded in the op lists.
G_MAT = np.array([
    [1, 0, 0],
    [1 / 2, 1 / 2, 1 / 2],
    [1 / 2, -1 / 2, 1 / 2],
    [0, 0, 1],
], dtype=np.float64)


def build_nc(npc=NPC, warmup_mms=30):
    nc = bacc.Bacc("TRN2", target_bir_lowering=False, debug=False)
    x_d = nc.dram_tensor("x", [npc, C, H, W], F32, kind="ExternalInput").ap()
    w_d = nc.dram_tensor("wu", [128, NU * 3 * KT * OT * 128], FP16,
                         kind="ExternalInput").ap()
    out_d = nc.dram_tensor("out", [npc, O, H, W], F32,
                           kind="ExternalOutput").ap()

    with tile.TileContext(nc) as tc, ExitStack() as ctx:
        wpool = ctx.enter_context(tc.tile_pool(name="wpool", bufs=1))
        xspool = ctx.enter_context(tc.tile_pool(name="xspool", bufs=30))
        hpool = ctx.enter_context(tc.tile_pool(name="hpool", bufs=2))
        vpool = ctx.enter_context(tc.tile_pool(name="vpool", bufs=2))
        s32pool = ctx.enter_context(tc.tile_pool(name="s32pool", bufs=10))
        opool = ctx.enter_context(tc.tile_pool(name="opool", bufs=6))
        ppool = ctx.enter_context(tc.tile_pool(name="ppool", bufs=8,
                                               space="PSUM"))

        w_sb = wpool.tile([128, NU * 3 * KT * OT * 128], FP16)
        nc.scalar.dma_start(w_sb[:, :], w_d[:, :])

        def wslice(u, kw, kt, ot):
            j = (((u * 3 + kw) * KT + kt) * OT + ot) * 128
            return w_sb[:, j:j + 128]

        if warmup_mms:
            # steady-state-shaped warmup: FD=392 MMs (LDW per MM, like the
            # real stream); memset on gpsimd, whose program loads earliest
            wu = wpool.tile([128, 392], FP16)
            nc.gpsimd.memset(wu[:, :], 0.0)
            wu_ps = ppool.tile([128, 512], F32, tag="ps", name="wu_ps")
            for i in range(warmup_mms):
                nc.tensor.matmul(wu_ps[:, :392], wu[:, :128],
                                 wu[:, :].rearrange(
                                     "p (t c) -> p t c", c=56),
                                 start=True, stop=True)

        xp_t = {}  # img -> xp tile ([128, 2*PLANE] fp16)
        vv = {}    # img -> [128, 2, NU, TW, WP] view

        def emit_pad(img):
            xp = hpool.tile([128, KT * PLANE], FP16, tag="xp")
            xp_t[img] = xp
            for kt in range(KT):
                b = kt * PLANE
                nc.vector.memset(xp[:, b: b + WP + 1], 0.0)
                nc.vector.memset(xp[:, b + (HP - 1) * WP - 1: b + HP * WP],
                                 0.0)
                nc.vector.memset(
                    xp[:, b + 2 * WP - 1: b + 2 * WP - 1 + (HP - 3) * WP]
                    .rearrange("p (a b) -> p a b", b=WP)[:, :, 0:2], 0.0)

        def emit_strip_triggers(img, s0, s1, all_sync=False):
            tiles = []
            for kt in range(KT):
                for s in range(s0, s1):
                    xs = xspool.tile([128, 8, W], F32, tag="xs")
                    q = nc.sync if (kt == 0 or all_sync) else nc.scalar
                    q.dma_start(xs[:, :, :],
                                x_d[img, kt * 128:(kt + 1) * 128,
                                    8 * s: 8 * s + 8, :])
                    tiles.append((kt, s, xs))
            return tiles

        def emit_converts(img, tiles, eng=None):
            v = xp_t[img][:, :].rearrange("p (k r c) -> p k r c", k=KT, c=WP)
            for kt, s, xs in tiles:
                dst = v[:, kt, 1 + 8 * s: 9 + 8 * s, 1:1 + W]
                if eng == "dve":
                    nc.vector.tensor_copy(dst, xs[:, :, :])
                else:
                    nc.scalar.copy(dst, xs[:, :, :])

        V_OPS = {1: ("add", 1, 2), 2: ("sub", 2, 1),
                 0: ("sub", 0, 2), 3: ("sub", 1, 3)}

        def emit_V(img, half, per_kt=False, us=(1, 2, 0, 3)):
            """V planes (taps in `us`) for tiles [half*14, half*14+14)."""
            if img not in vv:
                vt = vpool.tile([128, KT * NU * TW * WP], FP16, tag="v")
                vv[img] = vt[:, :].rearrange(
                    "p (k u t c) -> p k u t c", k=KT, u=NU, c=WP)
            v5 = vv[img]
            ta, tb = half * 14, half * 14 + 14
            xp2 = xp_t[img][:, :].rearrange(
                "p (k t f c) -> p k t f c", k=KT, f=2, c=WP)

            kts = [(kt, kt + 1) for kt in range(KT)] if per_kt \
                else [(0, KT)]

            def R(ka, kb, i):
                q, rr = divmod(i, 2)
                return xp2[:, ka:kb, ta + q: tb + q, rr, :]

            # op-major across kt, in U_ORDER, so img-0 MM groups (which
            # consume both kt planes of one u) start as early as possible
            for u in us:
                kind, i0, i1 = V_OPS[u]
                op = nc.vector.tensor_add if kind == "add" \
                    else nc.vector.tensor_sub
                for ka, kb in kts:
                    op(v5[:, ka:kb, u, ta:tb, :],
                       R(ka, kb, i0), R(ka, kb, i1))

        def s32():
            t = s32pool.tile([128, TR * W], F32, tag="s32")
            return t[:, :].rearrange("p (t c) -> p t c", c=W)

        def emit_chunk(img, ci, ot, last=False, v_hook=None):
            m = {}
            ob = opool.tile([128, 2 * TR * W], F32, tag="ob")
            ob2 = ob[:, :].rearrange("p (t f c) -> p t f c", f=2, c=W)
            t1 = t2 = None
            dst = out_d[img, ot * 128:(ot + 1) * 128,
                        2 * TR * ci: 2 * TR * (ci + 1), :]
            dst2 = dst.rearrange("p (t f) c -> p t f c", f=2)
            for u in ((1, 2, 3, 0) if last else U_ORDER):
                if v_hook is not None:
                    v_hook(u)
                ps = ppool.tile([128, 512], F32, tag="ps", name=f"ps{u}")
                mv = ps[:, : TR * W].rearrange("p (t c) -> p t c", c=W)
                m[u] = mv
                idx = 0
                for kt in range(KT):
                    vsl = vv[img][:, kt, u, ci * TR:(ci + 1) * TR, :]
                    for kw in range(3):
                        nc.tensor.matmul(
                            mv[:, :, :], wslice(u, kw, kt, ot),
                            vsl[:, :, kw: kw + W],
                            start=(idx == 0), stop=(idx == 3 * KT - 1),
                        )
                        idx += 1
                # drains trail each accumulation group. TT is DVE/Pool-only,
                # Pool can't read PSUM, TT reads <=1 PSUM operand, so ACT
                # activation-copies bridge PSUM->SBUF for the second inputs.
                if u == 2:
                    c2 = s32()
                    nc.scalar.copy(c2, m[2])
                    t1, t2 = s32(), s32()
                    nc.vector.tensor_add(t1, m[1], c2)   # m1 + m2
                    nc.vector.tensor_sub(t2, m[1], c2)   # m1 - m2
                if u == 3 and last:
                    c3 = s32()
                    nc.scalar.copy(c3, m[3])
                    nc.vector.tensor_sub(ob2[:, :, 1, :], t2, c3)
                    nc.sync.dma_start(dst2[:, :, 1, :], ob2[:, :, 1, :])
                if u == 0:
                    c0 = s32()
                    nc.scalar.copy(c0, m[0])
                    oeng = nc.vector if img == npc - 1 else nc.gpsimd
                    oeng.tensor_add(ob2[:, :, 0, :], t1, c0)
                    if last:
                        nc.sync.dma_start(dst2[:, :, 0, :], ob2[:, :, 0, :])
            if not last:
                c3 = s32()
                nc.scalar.copy(c3, m[3])
                oeng = nc.vector if img == npc - 1 else nc.gpsimd
                oeng.tensor_sub(ob2[:, :, 1, :], t2, c3)
                obv = ob[:, :].rearrange("p (r c) -> p r c", c=W)
                nc.sync.dma_start(dst[:, :, :], obv[:, :, :])

        # ---------------- schedule ----------------
        # In-order engines: DMA-paced prefetch ops must never be queued
        # ahead of ring-critical drain ops (ACT copies / DVE t1,t2), or the
        # PSUM ring stalls the PE.  So prefetch is sliced into small batches
        # emitted AFTER each chunk, each batch data-ready by its slot time.
        # image 0: kt1 converts ride gpsimd so both kt halves convert in
        # parallel; V per-kt so the first MM groups start early.
        emit_pad(0)
        st0 = emit_strip_triggers(0, 0, 4, all_sync=True)
        emit_converts(0, [t for t in st0 if t[0] == 0])
        emit_converts(0, [t for t in st0 if t[0] == 1], eng="dve")
        st1 = emit_strip_triggers(0, 4, NSTRIP, all_sync=True)

        CHUNKS = [(ci, ot) for ci in range(NCI) for ot in range(OT)]

        def cpick(strips, kt, ss):
            return [t for t in strips if t[0] == kt and t[1] in ss]

        for img in range(npc):
            nxt = img + 1
            if nxt < npc:
                emit_pad(nxt)
                strips = emit_strip_triggers(nxt, 0, NSTRIP)
            for qi, (ci, ot) in enumerate(CHUNKS):
                hook = None
                if img == 0 and qi == 0:
                    hook = lambda u: emit_V(0, 0, per_kt=True, us=(u,))
                emit_chunk(img, ci, ot,
                           last=(img == npc - 1 and qi == len(CHUNKS) - 1),
                           v_hook=hook)
                if img == 0:
                    # image 0's own half-1 input path
                    if qi == 0:
                        emit_converts(0, st1)
                    elif qi == 1:
                        emit_V(0, 1, us=(1, 2))
                    elif qi == 2:
                        emit_V(0, 1, us=(0, 3))
                    if nxt < npc:
                        if qi == 3:
                            emit_converts(nxt, cpick(strips, 0, (0, 1, 2)))
                        elif qi == 4:
                            emit_converts(nxt, cpick(strips, 0, (3, 4, 5)))
                        elif qi == 5:
                            emit_converts(nxt, cpick(strips, 0, (6,))
                                          + cpick(strips, 1, (0, 1, 2)))
                        elif qi == 6:
                            emit_converts(nxt, cpick(strips, 1, (3, 4, 5, 6)))
                            emit_V(nxt, 0, us=(1, 2))
                        elif qi == 7:
                            emit_V(nxt, 0, us=(0, 3))
                else:
                    # own half-1 V (its converts all done last window)
                    if qi == 0:
                        emit_V(img, 1, us=(1, 2))
                    elif qi == 1:
                        emit_V(img, 1, us=(0, 3))
                    if nxt < npc:
                        if qi == 2:
                            emit_converts(nxt, cpick(strips, 0, (0, 1, 2)))
                        elif qi == 3:
                            emit_converts(nxt, cpick(strips, 0, (3, 4, 5)))
                        elif qi == 4:
                            emit_converts(nxt, cpick(strips, 0, (6,))
                                          + cpick(strips, 1, (0, 1)))
                        elif qi == 5:
                            emit_converts(nxt, cpick(strips, 1, (2, 3, 4)))
                        elif qi == 6:
                            emit_converts(nxt, cpick(strips, 1, (5, 6)))
                            emit_V(nxt, 0, us=(1, 2))
                        elif qi == 7:
                            emit_V(nxt, 0, us=(0, 3))

    nc.compile()
    return nc


def quantize_weights(w):
    """Match reference fake-quant in f32: returns wq = dequantized weights."""
    w = np.asarray(w, np.float32)
    amax = np.max(np.abs(w), axis=(1, 2, 3), keepdims=True).astype(np.float32)
    scale = np.maximum((amax / np.float32(QMAX)).astype(np.float32),
                       np.float32(SCALING_MIN_VAL)).astype(np.float32)
    q = np.clip(np.rint((w / scale).astype(np.float32)),
                -QMAX, QMAX).astype(np.float32)
    return (q * scale).astype(np.float32)


def pack_weights(wq):
    """wq [O,C,3,3] -> winograd U packed [128, (u,kw,kt,ot,o_loc)] fp16."""
    u4 = np.einsum("ur,ocrk->uock", G_MAT,
                   wq.astype(np.float64)).astype(np.float32)
    a = u4.reshape(NU, OT, 128, KT, 128, 3)      # [u, ot, o, kt, c, kw]
    p = a.transpose(4, 0, 5, 3, 1, 2)            # [c, u, kw, kt, ot, o]
    return np.ascontiguousarray(p).reshape(
        128, NU * 3 * KT * OT * 128).astype(np.float16)


_nc_cache = {}
LAST_RESULT = None  # BassKernelResults of the most recent kernel() call


def kernel(x, w):
    global LAST_RESULT
    x = np.ascontiguousarray(np.asarray(x, np.float32))
    w = np.asarray(w, np.float32)
    assert x.shape == (N, C, H, W) and w.shape == (O, C, KH, KW)

    w_host = pack_weights(quantize_weights(w))

    if "nc" not in _nc_cache:
        _nc_cache["nc"] = build_nc()
    nc = _nc_cache["nc"]

    in_maps = [
        {"x": np.ascontiguousarray(x[cid * NPC:(cid + 1) * NPC]),
         "wu": w_host}
        for cid in range(CORES)
    ]
    kwargs = {}
    trace_dir = os.environ.get("KERNEL_TRACE_DIR")
    if trace_dir:  # dev-harness profiling only; unset in normal use
        kwargs = {"trace": True, "tmpdir": trace_dir}
    res = run_bass_kernel_spmd(nc, in_maps, list(range(CORES)), **kwargs)
    LAST_RESULT = res
    return np.concatenate([res.results[cid]["out"] for cid in range(CORES)],
                          axis=0)


if __name__ == "__main__":
    rng = np.random.default_rng(0)
    x = rng.standard_normal((N, C, H, W), dtype=np.float32)
    w = rng.standard_normal((O, C, KH, KW), dtype=np.float32) * 0.05
    out = kernel(x, w)
    print("out", out.shape, out.dtype, float(np.abs(out).max()))
